# revision 11
# baseline (speedup 1.0000x reference)
"""GNN message-passing (segment-mean + 3-layer MLP) Trainium2 kernel.

Strategy (8 NeuronCores, SPMD, full inputs in / full output out):
  - Host: assign nodes to 800 blocks of 64 slots (degree-balanced snake) so
    every block's incoming-edge count fits 6 k-tiles of 128 edges.  Blocks
    0-99 -> core 0, etc.  Edges are bucketed per receiver block, pre-scaled
    by 1/deg(recv), cast to bf16, and laid out [eslot, ktile*feat] so
    per-chunk DMAs are large and contiguous.  One-hot scatter masks are 64
    columns wide (fp8), halving mask DMA vs 128-wide blocks.
  - Device per core: segment-mean as mask matmuls on the TensorEngine (6
    k-tiles x 8 blocks accumulated into one 512-col PSUM bank per chunk),
    then the 3-layer MLP over 512-node chunks in feature-major layout.
    Everything except PSUM/bias/mask is bf16: halves DMA bytes and enables
    fast-weight-load on the PE (f32r disables FWL).  Edge/mask slabs stream
    on the SP HWDGE ring in ~0.4-0.8 MB granules, 3 chunks deep; x/weights/
    outputs ride the ACT ring.  Output is written bf16 and upcast on host.
"""
import sys

sys.path.insert(0, "/opt/trn_rl_repo")

import numpy as np
import ml_dtypes

from concourse import bacc
import concourse.mybir as mybir
import concourse.tile as tile
from concourse.bass_utils import run_bass_kernel_spmd

# problem shape (hardcoded per contract)
N_NODES = 50000
N_EDGES = 600000
D = 128          # node/edge feature dim
DH = 512         # hidden dim
C = 8            # cores
W = 64           # node slots per block
BPC = 100        # node blocks per core
NB = C * BPC     # 800 blocks total
SLOTS = BPC * W  # 6400 node slots per core
T_BLK = 6        # edge k-tiles (128 edges) per block
TT = BPC * T_BLK   # k-tiles per core
CHUNKS = [8] * 12 + [4]  # blocks per MLP chunk (512/256 nodes)
PREF = 4         # chunks of edge-slab prefetch depth

F32 = mybir.dt.float32
BF16 = mybir.dt.bfloat16
FP8 = mybir.dt.float8e4

_prog_cache = {}
LAST_RESULTS = None  # BassKernelResults of the most recent run (for test.py)


def _build_program(t_blk=T_BLK):
    if t_blk in _prog_cache:
        return _prog_cache[t_blk]
    tt = BPC * t_blk
    nc = bacc.Bacc("TRN2", target_bir_lowering=False)

    xT_d = nc.declare_dram_parameter("xT", [128, SLOTS], BF16, isOutput=False)
    ea_d = nc.declare_dram_parameter("ea", [128, tt * D], BF16, isOutput=False)
    mk_d = nc.declare_dram_parameter("mk", [128, tt * W], FP8, isOutput=False)
    w1_d = nc.declare_dram_parameter("w1", [2 * D, DH], BF16, isOutput=False)
    w2_d = nc.declare_dram_parameter("w2", [DH, DH], BF16, isOutput=False)
    w3_d = nc.declare_dram_parameter("w3", [DH, D], BF16, isOutput=False)
    b1_d = nc.declare_dram_parameter("b1", [128, 4], F32, isOutput=False)
    b2_d = nc.declare_dram_parameter("b2", [128, 4], F32, isOutput=False)
    b3_d = nc.declare_dram_parameter("b3", [128, 1], F32, isOutput=False)
    out_d = nc.declare_dram_parameter("outT", [128, SLOTS], BF16, isOutput=True)

    RELU = mybir.ActivationFunctionType.Relu
    ADD = mybir.AluOpType.add

    n_chunks = len(CHUNKS)
    chunk_blk0 = [0]
    for c_i in range(1, n_chunks):
        chunk_blk0.append(chunk_blk0[-1] + CHUNKS[c_i - 1])

    with tile.TileContext(nc) as tc:
        with (
            tc.tile_pool(name="pers", bufs=1) as pers,
            tc.tile_pool(name="eap", bufs=13) as eap,
            tc.tile_pool(name="mkp", bufs=13) as mkp,
            tc.tile_pool(name="xp", bufs=6) as xp,
            tc.tile_pool(name="actp", bufs=2) as actp,
            tc.tile_pool(name="scat_ps", bufs=2, space="PSUM") as scat_ps,
            tc.tile_pool(name="mlp_ps", bufs=5, space="PSUM") as mlp_ps,
        ):
            # granules[(chunk, block)] = (ea_tile, mk_tile, local_block)
            gran = {}

            def load_granule(c_i, b0, nblk):
                kt0 = (chunk_blk0[c_i] + b0) * t_blk
                nkt = nblk * t_blk
                ea_t = eap.tile([128, 4 * t_blk * D], BF16, tag="ea")
                nc.sync.dma_start(
                    out=ea_t[:, : nkt * D], in_=ea_d[:, kt0 * D : (kt0 + nkt) * D]
                )
                mk_t = mkp.tile([128, 4 * t_blk * W], FP8, tag="mk")
                nc.sync.dma_start(
                    out=mk_t[:, : nkt * W], in_=mk_d[:, kt0 * W : (kt0 + nkt) * W]
                )
                for lb in range(nblk):
                    gran[(c_i, b0 + lb)] = (ea_t, mk_t, lb)

            def load_chunk_slabs(c_i, granule_blks):
                nb = CHUNKS[c_i]
                if isinstance(granule_blks, int):
                    granule_blks = [granule_blks] * ((nb + granule_blks - 1) // granule_blks)
                b0 = 0
                for g in granule_blks:
                    g = min(g, nb - b0)
                    if g <= 0:
                        break
                    load_granule(c_i, b0, g)
                    b0 += g

            x_ts = {}

            def load_x(c_i):
                ncw = CHUNKS[c_i] * W
                xt = xp.tile([128, 512], BF16, tag="x")
                nc.scalar.dma_start(
                    out=xt[:, :ncw],
                    in_=xT_d[:, chunk_blk0[c_i] * W : chunk_blk0[c_i] * W + ncw],
                )
                x_ts[c_i] = xt

            # --- edge/mask slabs stream on the SP ring; first chunk in ramped
            # granules so the first scatter matmul starts ASAP ---
            load_chunk_slabs(0, [1, 1, 2, 4])
            # --- persistent tiles + x on the ACT ring ---
            b1t = pers.tile([128, 4], F32)
            nc.scalar.dma_start(out=b1t[:], in_=b1_d[:])
            b2t = pers.tile([128, 4], F32)
            nc.scalar.dma_start(out=b2t[:], in_=b2_d[:])
            b3t = pers.tile([128, 1], F32)
            nc.scalar.dma_start(out=b3t[:], in_=b3_d[:])
            load_x(0)
            w1t = pers.tile([128, 2, DH], BF16)
            nc.scalar.dma_start(out=w1t[:], in_=w1_d[:].rearrange("(k p) m -> p k m", p=128))
            load_chunk_slabs(1, 4)
            load_x(1)
            w2t = pers.tile([128, 4, DH], BF16)
            nc.scalar.dma_start(out=w2t[:], in_=w2_d[:].rearrange("(k p) m -> p k m", p=128))
            load_chunk_slabs(2, 4)
            load_x(2)
            w3t = pers.tile([128, 4, D], BF16)
            nc.scalar.dma_start(out=w3t[:], in_=w3_d[:].rearrange("(k p) m -> p k m", p=128))
            load_chunk_slabs(3, 4)
            load_x(3)
            load_x(4)

            col0 = 0
            for c_i, nb in enumerate(CHUNKS):
                NCW = nb * W
                # scatter: segment-mean via mask matmuls, one PSUM bank per chunk
                ps = scat_ps.tile([128, 512], F32, tag="scat")
                for b in range(nb):
                    ea_t, mk_t, lb = gran.pop((c_i, b))
                    for t in range(t_blk):
                        j = (lb * t_blk + t)
                        nc.tensor.matmul(
                            out=ps[:, b * W : (b + 1) * W],
                            lhsT=ea_t[:, j * D : (j + 1) * D],
                            rhs=mk_t[:, j * W : (j + 1) * W],
                            start=(t == 0),
                            stop=(t == t_blk - 1),
                        )
                mean_t = actp.tile([128, 512], BF16, tag="mean")
                nc.vector.tensor_copy(out=mean_t[:, :NCW], in_=ps[:, :NCW])

                # prefetch a later chunk's slabs / x
                if c_i + PREF < n_chunks:
                    load_chunk_slabs(c_i + PREF, 4)
                if c_i + PREF + 1 < n_chunks:
                    load_x(c_i + PREF + 1)

                # layer 1: h1 = relu(W1.T @ [x; mean] + b1)
                h1_t = actp.tile([128, 4, 512], BF16, tag="h1")
                for m in range(4):
                    pm = mlp_ps.tile([128, 512], F32, tag="mlp")
                    nc.tensor.matmul(
                        out=pm[:, :NCW],
                        lhsT=w1t[:, 1, m * 128 : (m + 1) * 128],
                        rhs=mean_t[:, :NCW],
                        start=True,
                        stop=False,
                    )
                    nc.tensor.matmul(
                        out=pm[:, :NCW],
                        lhsT=w1t[:, 0, m * 128 : (m + 1) * 128],
                        rhs=x_ts[c_i][:, :NCW],
                        start=False,
                        stop=True,
                    )
                    if m % 2 == 0:
                        nc.scalar.activation(
                            out=h1_t[:, m, :NCW], in_=pm[:, :NCW], func=RELU, bias=b1t[:, m : m + 1]
                        )
                    else:
                        nc.vector.tensor_scalar(
                            out=h1_t[:, m, :NCW], in0=pm[:, :NCW], scalar1=b1t[:, m : m + 1],
                            scalar2=0.0, op0=ADD, op1=mybir.AluOpType.max,
                        )
                # layer 2
                h2_t = actp.tile([128, 4, 512], BF16, tag="h2")
                for m in range(4):
                    pm = mlp_ps.tile([128, 512], F32, tag="mlp")
                    for k in range(4):
                        nc.tensor.matmul(
                            out=pm[:, :NCW],
                            lhsT=w2t[:, k, m * 128 : (m + 1) * 128],
                            rhs=h1_t[:, k, :NCW],
                            start=(k == 0),
                            stop=(k == 3),
                        )
                    if m % 2 == 0:
                        nc.scalar.activation(
                            out=h2_t[:, m, :NCW], in_=pm[:, :NCW], func=RELU, bias=b2t[:, m : m + 1]
                        )
                    else:
                        nc.vector.tensor_scalar(
                            out=h2_t[:, m, :NCW], in0=pm[:, :NCW], scalar1=b2t[:, m : m + 1],
                            scalar2=0.0, op0=ADD, op1=mybir.AluOpType.max,
                        )
                # layer 3: out = W3.T @ h2 + b3  (bias add on DVE, bf16 out).
                # The last chunk's output is written in two halves so the
                # final bias-add + DMA + completion chain is short.
                out_t = actp.tile([128, 512], BF16, tag="out")
                halves = [(0, NCW)] if c_i + 1 < n_chunks else [(0, NCW // 2), (NCW // 2, NCW)]
                for h0, h1 in halves:
                    pm = mlp_ps.tile([128, 512], F32, tag="mlp")
                    for k in range(4):
                        nc.tensor.matmul(
                            out=pm[:, h0:h1],
                            lhsT=w3t[:, k, :],
                            rhs=h2_t[:, k, h0:h1],
                            start=(k == 0),
                            stop=(k == 3),
                        )
                    nc.vector.tensor_scalar_add(out_t[:, h0:h1], pm[:, h0:h1], b3t[:, 0:1])
                    nc.scalar.dma_start(
                        out=out_d[:, col0 + h0 : col0 + h1], in_=out_t[:, h0:h1]
                    )
                col0 += NCW

    nc.compile()
    _prog_cache[t_blk] = nc
    return nc


def _preprocess(x, edge_index, edge_attr):
    recv = np.asarray(edge_index)[1].astype(np.int64)
    deg = np.bincount(recv, minlength=N_NODES)
    # snake assignment of degree-sorted nodes into NB blocks (62-63 nodes each)
    order = np.argsort(-deg, kind="stable")
    i = np.arange(N_NODES)
    rnd, pos = i // NB, i % NB
    blk = np.where(rnd % 2 == 0, pos, NB - 1 - pos)
    node_block = np.empty(N_NODES, np.int64)
    node_slot = np.empty(N_NODES, np.int64)
    node_block[order] = blk
    node_slot[order] = rnd
    node_core = node_block // BPC
    node_col = (node_block % BPC) * W + node_slot

    eb = node_block[recv]
    bc = np.bincount(eb, minlength=NB)
    t_blk = max(T_BLK, int(-(-int(bc.max()) // 128)))  # >= ceil(max_load/128)
    tt = BPC * t_blk

    eorder = np.argsort(eb, kind="stable")
    eb_s = eb[eorder]
    starts = np.zeros(NB, np.int64)
    starts[1:] = np.cumsum(bc)[:-1]
    ewithin = np.arange(N_EDGES) - starts[eb_s]
    ktile = ewithin // 128
    eslot = ewithin % 128
    ecore = eb_s // BPC
    kt_in_core = (eb_s % BPC) * t_blk + ktile

    # scale edges by 1/deg(recv) on the host, then cast once to bf16
    ea_scaled = np.asarray(edge_attr, np.float32) * (1.0 / deg[recv])[:, None].astype(np.float32)
    ea_bf = ea_scaled.astype(ml_dtypes.bfloat16)
    ea_buf = np.zeros((C, tt, 128, D), ml_dtypes.bfloat16)
    ea_buf[ecore, kt_in_core, eslot] = ea_bf[eorder]
    # 0/1 scatter masks in fp8, 64 columns per block
    mk_buf = np.zeros((C, tt, 128, W), ml_dtypes.float8_e4m3)
    mk_buf[ecore, kt_in_core, eslot, (node_col[recv] % W)[eorder]] = 1.0

    X_all = np.zeros((C, SLOTS, D), ml_dtypes.bfloat16)
    X_all[node_core, node_col] = np.asarray(x, np.float32).astype(ml_dtypes.bfloat16)

    shards = []
    for c in range(C):
        shards.append(
            dict(
                xT=np.ascontiguousarray(X_all[c].T),
                ea=np.ascontiguousarray(ea_buf[c].transpose(1, 0, 2).reshape(128, tt * D)),
                mk=np.ascontiguousarray(mk_buf[c].transpose(1, 0, 2).reshape(128, tt * W)),
            )
        )
    return shards, node_core, node_col, t_blk


def kernel(x, edge_index, edge_attr, W1, b1, W2, b2, W3, b3, _trace=False):
    global LAST_RESULTS
    shards, node_core, node_col, t_blk = _preprocess(x, edge_index, edge_attr)

    W1 = np.ascontiguousarray(np.asarray(W1, np.float32).astype(ml_dtypes.bfloat16))
    W2 = np.ascontiguousarray(np.asarray(W2, np.float32).astype(ml_dtypes.bfloat16))
    W3 = np.ascontiguousarray(np.asarray(W3, np.float32).astype(ml_dtypes.bfloat16))
    b1r = np.ascontiguousarray(np.asarray(b1, np.float32).reshape(4, 128).T)
    b2r = np.ascontiguousarray(np.asarray(b2, np.float32).reshape(4, 128).T)
    b3r = np.ascontiguousarray(np.asarray(b3, np.float32).reshape(1, 128).T)

    in_maps = []
    for c in range(C):
        m = dict(shards[c])
        m.update(w1=W1, w2=W2, w3=W3, b1=b1r, b2=b2r, b3=b3r)
        in_maps.append(m)

    nc = _build_program(t_blk)
    res = run_bass_kernel_spmd(nc, in_maps, core_ids=list(range(C)), trace=_trace)
    LAST_RESULTS = res

    outs = np.stack([res.results[c]["outT"] for c in range(C)])  # [C, 128, SLOTS] bf16
    out = outs.transpose(0, 2, 1)[node_core, node_col]
    return np.ascontiguousarray(out, dtype=np.float32)
